# revision 1
# baseline (speedup 1.0000x reference)
"""Trainium2 Bass kernel for nn_Block_34711925686740 (MLA attention + DeepSeekMoE).

Sharding (8 NeuronCores, SPMD single program, all per-core differences via data):
  core c -> batch row b=c//2, token half h=c%2 (512 "own" tokens).
  Attention: q over own 512 tokens vs the full row's 1024 keys; causality applied
  with a per-core multiplicative 0/1 mask input (keeps the program uniform).
  Router/MoE/shared experts: token-parallel over own 512 tokens, dense experts
  weighted by routing weights (zero weight for unselected -> exact).

Device layout: activations feature-major [feature, token]; matmuls contract over
the partition dim. Big matmuls run in bf16 (fp32 PSUM accumulate); softmax,
norms, router and residual paths stay fp32/float32r. Weights are pre-tiled on
host to [M/512, K/128, 128, 512] bf16 so each SBUF weight tile is one DMA with
1KB contiguous lines.

Host folds: g_attn/g_moe into consuming weights, 1/sqrt(HD) into wq,
1/sqrt(C) into wr. Output is assembled (transpose + concat) on host.
"""
import contextlib
import sys

sys.path.insert(0, "/opt/trn_rl_repo")

import ml_dtypes
import numpy as np

import concourse.bass as bass
import concourse.mybir as mybir
import concourse.tile as tile
from concourse import bacc
from concourse.bass_utils import run_bass_kernel_spmd
from concourse.masks import make_identity

FP32 = mybir.dt.float32
FP32R = mybir.dt.float32r
BF16 = mybir.dt.bfloat16

B, T, C = 4, 1024, 1024
H, HD, LAT = 16, 64, 512
E, KTOP, F = 8, 2, 1024
NSH = 2
F2 = F * NSH
EPS = 1e-6
P = 128
OWN = 512          # tokens owned per core
ROW = 1024         # tokens in the core's batch row
KC = C // P        # 8 k-tiles over C
MCH = 512          # weight m-chunk (columns per pre-tiled chunk)
NCORES = 8

Act = mybir.ActivationFunctionType
AxX = mybir.AxisListType.X


def _r(ap):
    """DRAM [K, M] -> [p, ko, m] partition-inner view."""
    return ap.rearrange("(ko p) m -> p ko m", p=P)


def build_nc(debug=False):
    nc = bacc.Bacc("TRN2", target_bir_lowering=False, debug=False,
                   num_devices=NCORES)

    def din(name, shape, dt=FP32):
        return nc.dram_tensor(name, shape, dt, kind="ExternalInput").ap()

    # per-core activations
    x_rowT = din("x_rowT", [C, ROW])
    x_ownT = din("x_ownT", [C, OWN])
    maskT = din("maskT", [ROW, OWN], BF16)
    # weights, bf16 pre-tiled [mo, ko, P, mch] (g/scales folded on host)
    def wtiled(name, kdim, mdim, nmat=None):
        mo = (mdim + MCH - 1) // MCH
        shape = [mo, kdim // P, P, min(MCH, mdim)]
        if nmat is not None:
            shape = [nmat] + shape
        return din(name, shape, BF16)

    wq = wtiled("wq", C, H * HD)
    wkv = wtiled("wkv", C, LAT)
    wk_up = wtiled("wk_up", LAT, HD)
    wv_up = wtiled("wv_up", LAT, HD)
    wo = wtiled("wo", H * HD, C)
    e_w1 = wtiled("e_w1", C, F, E)
    e_w2 = wtiled("e_w2", F, C, E)
    e_w3 = wtiled("e_w3", C, F, E)
    s_w1 = wtiled("s_w1", C, F2)
    s_w2 = wtiled("s_w2", F2, C)
    s_w3 = wtiled("s_w3", C, F2)
    wr = din("wr", [C, E])
    rb = din("rb", [E, 1])

    outT = nc.dram_tensor("outT", [C, OWN], FP32, kind="ExternalOutput").ap()
    dbg = {}
    if debug:
        for name, shape, dt in [
            ("d_xn_own", [C, OWN], BF16), ("d_qT", [H * HD, OWN], BF16),
            ("d_kT", [HD, ROW], BF16), ("d_yT", [H * HD, OWN], BF16),
            ("d_hT", [C, OWN], FP32), ("d_w", [E, OWN], FP32),
            ("d_logits", [E, OWN], FP32), ("d_moe", [C, OWN], FP32),
        ]:
            dbg[name] = nc.dram_tensor(name, shape, dt,
                                       kind="ExternalOutput").ap()

    with tile.TileContext(nc) as tc:
        _build_body(nc, tc, locals(), dbg, debug)
    nc.compile()
    return nc


def _build_body(nc, tc, T_, dbg, debug):
    x_rowT, x_ownT, maskT = T_["x_rowT"], T_["x_ownT"], T_["maskT"]
    wq, wkv, wk_up, wv_up, wo = T_["wq"], T_["wkv"], T_["wk_up"], T_["wv_up"], T_["wo"]
    wr, rb = T_["wr"], T_["rb"]
    e_w1, e_w2, e_w3 = T_["e_w1"], T_["e_w2"], T_["e_w3"]
    s_w1, s_w2, s_w3 = T_["s_w1"], T_["s_w2"], T_["s_w3"]
    outT = T_["outT"]

    out_es = contextlib.ExitStack()
    with out_es:
        const = out_es.enter_context(tc.tile_pool(name="const", bufs=1))
        wpool = out_es.enter_context(tc.tile_pool(name="wpool", bufs=4))
        hold = out_es.enter_context(tc.tile_pool(name="hold", bufs=1))

        ident = const.tile([P, P], FP32)
        make_identity(nc, ident)
        ident_b = const.tile([P, P], BF16)
        nc.vector.tensor_copy(ident_b, ident)
        ones_f = const.tile([P, 1], FP32)
        nc.vector.memset(ones_f, 1.0)
        ones_b = const.tile([P, 1], BF16)
        nc.vector.tensor_copy(ones_b, ones_f)
        rb_sb = const.tile([E, 1], FP32)
        nc.sync.dma_start(rb_sb, rb)
        eps1 = const.tile([1, 1], FP32)
        nc.vector.memset(eps1, EPS)

        def dbg_dump(name, src_ap, shape3=True):
            if not debug:
                return
            dst = _r(dbg[name]) if shape3 else dbg[name]
            if src_ap.dtype == FP32R:
                src_ap = src_ap.bitcast(FP32)
            nc.sync.dma_start(dst, src_ap)

        def load_w(w_ap, m2, kgroup=None):
            """One pre-tiled weight chunk -> SBUF [P, <=8, mch] bf16 tile."""
            src = w_ap[m2]
            if kgroup is not None:
                src = src[kgroup * KC:(kgroup + 1) * KC]
            ko, _, mch = src.shape
            t = wpool.tile([P, KC, MCH], BF16, tag="wtile")
            nc.sync.dma_start(t[:, :ko, :mch],
                              src.rearrange("ko p m -> p ko m"))
            return t

        # ================= rmsnorm (feature-major) =================
        def rmsnorm(src, ntok, dst_pool, out_tag, ps_pool, sc_pool,
                    dt=BF16):
            """src [P, KC, ntok] fp32 -> normalized [P, KC, ntok] in dt."""
            ssq = ps_pool.tile([1, ntok], FP32, tag="rms_ps")
            for k in range(KC):
                sq = sc_pool.tile([P, ntok], BF16, tag="rms_sq")
                nc.vector.tensor_mul(sq, src[:, k], src[:, k])
                for no in range(ntok // 512):
                    nc.tensor.matmul(ssq[:, no * 512:(no + 1) * 512], ones_b,
                                     sq[:, no * 512:(no + 1) * 512],
                                     start=(k == 0), stop=(k == KC - 1))
            srow = sc_pool.tile([1, ntok], FP32, tag="rms_srow")
            nc.scalar.activation(srow, ssq, Act.Sqrt, scale=1.0 / C, bias=eps1)
            rrow = sc_pool.tile([1, ntok], FP32, tag="rms_rrow")
            nc.vector.reciprocal(rrow, srow)
            bc = sc_pool.tile([P, ntok], FP32, tag="rms_bc")
            nc.gpsimd.partition_broadcast(bc, rrow)
            dst = dst_pool.tile([P, KC, ntok], dt, tag=out_tag)
            for k in range(KC):
                nc.vector.tensor_mul(dst[:, k], src[:, k], bc)
            return dst

        xown = hold.tile([P, KC, OWN], FP32, tag="xown")
        nc.sync.dma_start(xown, _r(x_ownT))

        es_n = contextlib.ExitStack()
        pool_n = es_n.enter_context(tc.tile_pool(name="pool_n", bufs=1))
        with contextlib.ExitStack() as es_x:
            pool_x = es_x.enter_context(
                tc.tile_pool(name="pool_x", bufs=1, side="right"))
            sc_1 = es_x.enter_context(tc.tile_pool(name="sc_1", bufs=2))
            ps_1 = es_x.enter_context(
                tc.tile_pool(name="ps_1", bufs=2, space="PSUM"))
            xrow = pool_x.tile([P, KC, ROW], FP32, tag="xrow")
            nc.sync.dma_start(xrow, _r(x_rowT))
            xn_own = rmsnorm(xown, OWN, pool_n, "xn_own", ps_1, sc_1)
            xn_row = rmsnorm(xrow, ROW, pool_n, "xn_row", ps_1, sc_1)
        dbg_dump("d_xn_own", xn_own)
        es_att = contextlib.ExitStack()
        pool_att = es_att.enter_context(
            tc.tile_pool(name="pool_att", bufs=1, side="right"))
        es_kv = contextlib.ExitStack()
        pool_kv = es_kv.enter_context(
            tc.tile_pool(name="pool_kv", bufs=1, side="right"))

        # ================= projections =================
        def mm_project(w_ap, kdim, mdim, rhs, ntok, out_pool, out_tag, ps_pool,
                       out_dt=BF16):
            """out[mdim, ntok] = w.T @ rhs, bf16 operands, out in out_dt."""
            ko = kdim // P
            mo = (mdim + P - 1) // P
            out = out_pool.tile([P, mo, ntok], out_dt, tag=out_tag)
            for m2 in range((mdim + MCH - 1) // MCH):
                wt = load_w(w_ap, m2)
                mch = min(MCH, mdim - m2 * MCH)
                for ms in range((mch + P - 1) // P):
                    m = m2 * (MCH // P) + ms
                    mt = min(P, mdim - m * P)
                    for no in range(ntok // 512):
                        psum = ps_pool.tile([P, 512], FP32, tag="proj_ps")
                        for k in range(ko):
                            nc.tensor.matmul(
                                psum[:mt], wt[:, k, ms * P:ms * P + mt],
                                rhs[:, k, no * 512:(no + 1) * 512],
                                start=(k == 0), stop=(k == ko - 1))
                        nc.vector.tensor_copy(
                            out[:mt, m, no * 512:(no + 1) * 512], psum[:mt])
            return out

        with contextlib.ExitStack() as es_p:
            ps_2 = es_p.enter_context(
                tc.tile_pool(name="ps_2", bufs=3, space="PSUM"))
            qT = mm_project(wq, C, H * HD, xn_own, OWN, pool_att, "qT", ps_2)
            kvT = mm_project(wkv, C, LAT, xn_row, ROW, pool_kv, "kvT", ps_2)
        es_n.close()  # frees xn_row / xn_own

        # kT duplicated into both partition halves so lhsT base matches q_h base
        kdup = pool_att.tile([P, ROW], BF16, tag="kdup")
        with contextlib.ExitStack() as es_p:
            ps_3 = es_p.enter_context(
                tc.tile_pool(name="ps_3", bufs=2, space="PSUM"))
            kT = mm_project(wk_up, LAT, HD, kvT, ROW, pool_kv, "kT", ps_3)
            vT = mm_project(wv_up, LAT, HD, kvT, ROW, pool_kv, "vT", ps_3)
            # v token-major [ROW, HD]
            v_tm = pool_att.tile([P, ROW // P, HD], BF16, tag="v_tm")
            for j in range(ROW // P):
                tp = ps_3.tile([P, HD], BF16, tag="vtp")
                nc.tensor.transpose(tp, vT[:HD, 0, j * P:(j + 1) * P],
                                    ident_b[:HD, :HD])
                nc.vector.tensor_copy(v_tm[:, j, :], tp)
            nc.vector.tensor_copy(kdup[:HD, :], kT[:HD, 0, :])
            nc.sync.dma_start(kdup[64:64 + HD, :], kT[:HD, 0, :])
            if debug:
                nc.sync.dma_start(dbg["d_kT"], kT[:HD, 0, :])
        es_kv.close()
        dbg_dump("d_qT", qT)

        # ================= attention core =================
        mask_sb = pool_att.tile([P, ROW // P, OWN], BF16, tag="mask")
        nc.sync.dma_start(mask_sb, _r(maskT))
        yT = pool_att.tile([P, H * HD // P, OWN], BF16, tag="yT")
        SJ = ROW // P
        # software pipeline: head hh's exp tiles are produced while head
        # hh-1's Z/y accumulations drain, so the PE never waits on ACT/DVE.
        with contextlib.ExitStack() as es_p:
            ps_sc = es_p.enter_context(
                tc.tile_pool(name="ps_sc", bufs=4, space="PSUM"))
            ps_zy = es_p.enter_context(
                tc.tile_pool(name="ps_zy", bufs=2, space="PSUM"))
            sc = es_p.enter_context(
                tc.tile_pool(name="sc_att", bufs=4, side="right"))
            ebpool = es_p.enter_context(
                tc.tile_pool(name="ebpool", bufs=18, side="right"))

            def head_scores(hh):
                p2 = 64 * (hh % 2)
                q_h = qT[p2:p2 + 64, hh // 2, :]
                ebs = []
                for j in range(SJ):
                    sc_ps = ps_sc.tile([P, OWN], FP32, tag="sc_ps")
                    nc.tensor.matmul(sc_ps, kdup[p2:p2 + HD, j * P:(j + 1) * P],
                                     q_h, start=True, stop=True)
                    e_sb = sc.tile([P, OWN], BF16, tag="e_sb")
                    nc.scalar.activation(e_sb, sc_ps, Act.Exp)
                    e_b = ebpool.tile([P, OWN], BF16, tag="e_b")
                    nc.vector.tensor_mul(e_b, e_sb, mask_sb[:, j, :])
                    ebs.append(e_b)
                return ebs

            def head_drain(hh, ebs):
                z_ps = ps_zy.tile([1, OWN], FP32, tag="z_ps")
                y_ps = ps_zy.tile([64, OWN], FP32, tag="y_ps")
                for j in range(SJ):
                    nc.tensor.matmul(z_ps, ones_b, ebs[j],
                                     start=(j == 0), stop=(j == SJ - 1))
                    nc.tensor.matmul(y_ps, v_tm[:, j, :], ebs[j],
                                     start=(j == 0), stop=(j == SJ - 1))
                rz = sc.tile([1, OWN], FP32, tag="rz")
                nc.vector.reciprocal(rz, z_ps)
                zbc = sc.tile([64, OWN], FP32, tag="zbc")
                nc.gpsimd.partition_broadcast(zbc, rz)
                if hh % 2 == 0:
                    nc.vector.tensor_mul(yT[:64, hh // 2, :], y_ps, zbc)
                else:
                    ynorm = sc.tile([64, OWN], BF16, tag="ynorm")
                    nc.vector.tensor_mul(ynorm, y_ps, zbc)
                    nc.sync.dma_start(yT[64:128, hh // 2, :], ynorm)

            prev = None
            for hh in range(H):
                ebs = head_scores(hh)
                if prev is not None:
                    head_drain(prev[0], prev[1])
                prev = (hh, ebs)
            head_drain(prev[0], prev[1])
        dbg_dump("d_yT", yT)

        # ================= wo + residual =================
        hT = hold.tile([P, KC, OWN], FP32, tag="hT")
        with contextlib.ExitStack() as es_p:
            ps_wo = es_p.enter_context(
                tc.tile_pool(name="ps_wo", bufs=2, space="PSUM"))
            for m2 in range(C // MCH):
                wt = load_w(wo, m2)
                for ms in range(MCH // P):
                    cm = m2 * (MCH // P) + ms
                    psum = ps_wo.tile([P, OWN], FP32, tag="wo_ps")
                    for k in range(KC):
                        nc.tensor.matmul(psum, wt[:, k, ms * P:(ms + 1) * P],
                                         yT[:, k, :], start=(k == 0),
                                         stop=(k == KC - 1))
                    nc.vector.tensor_add(hT[:, cm, :], psum, xown[:, cm, :])
        es_att.close()
        dbg_dump("d_hT", hT)

        # ================= MoE norm + router =================
        es_moe = contextlib.ExitStack()
        pool_moe = es_moe.enter_context(tc.tile_pool(name="pool_moe", bufs=1))
        hidpool = es_moe.enter_context(tc.tile_pool(name="hidpool", bufs=2))
        with contextlib.ExitStack() as es_p:
            ps_5 = es_p.enter_context(
                tc.tile_pool(name="ps_5", bufs=2, space="PSUM"))
            sc = es_p.enter_context(tc.tile_pool(name="sc_rt", bufs=3))
            # fp32r copy for the router (selection is precision-sensitive),
            # bf16 copy for the expert/shared matmuls
            hnR = rmsnorm(hT, OWN, pool_moe, "hnR", ps_5, sc, dt=FP32R)
            hnT = pool_moe.tile([P, KC, OWN], BF16, tag="hnT")
            nc.vector.tensor_copy(hnT, hnR)

            lg_ps = ps_5.tile([E, OWN], FP32, tag="lg_ps")
            wr_sb = const.tile([P, KC, E], FP32R)
            nc.sync.dma_start(wr_sb, _r(wr).bitcast(FP32R))
            for k in range(KC):
                nc.tensor.matmul(lg_ps, wr_sb[:, k, :], hnR[:, k, :],
                                 start=(k == 0), stop=(k == KC - 1))
            logitsT = pool_moe.tile([E, OWN], FP32, tag="logitsT")
            nc.vector.tensor_copy(logitsT, lg_ps)
            biasedT = pool_moe.tile([E, OWN], FP32, tag="biasedT")
            nc.vector.tensor_scalar_add(biasedT, logitsT, rb_sb)

            TC4 = OWN // P
            w_tm = pool_moe.tile([P, TC4, E], FP32, tag="w_tm")
            for t4 in range(TC4):
                bt_ps = ps_5.tile([P, E], FP32, tag="rt_ps")
                nc.tensor.transpose(bt_ps, biasedT[:, t4 * P:(t4 + 1) * P],
                                    ident[:E, :E])
                bt = sc.tile([P, E], FP32, tag="bt")
                nc.vector.tensor_copy(bt, bt_ps)
                lt_ps = ps_5.tile([P, E], FP32, tag="rt_ps")
                nc.tensor.transpose(lt_ps, logitsT[:, t4 * P:(t4 + 1) * P],
                                    ident[:E, :E])
                top8 = sc.tile([P, 8], FP32, tag="top8")
                nc.vector.max(out=top8, in_=bt)
                sel = sc.tile([P, E], FP32, tag="sel")
                nc.vector.tensor_scalar(sel, bt, top8[:, KTOP - 1:KTOP], None,
                                        op0=mybir.AluOpType.is_ge)
                expl = sc.tile([P, E], FP32, tag="expl")
                nc.scalar.activation(expl, lt_ps, Act.Exp)
                msk = sc.tile([P, E], FP32, tag="msk")
                nc.vector.tensor_mul(msk, expl, sel)
                zsum = sc.tile([P, 1], FP32, tag="zsum")
                nc.vector.reduce_sum(zsum, msk, axis=AxX)
                rzs = sc.tile([P, 1], FP32, tag="rzs")
                nc.vector.reciprocal(rzs, zsum)
                nc.vector.tensor_scalar_mul(w_tm[:, t4, :], msk, rzs)

            wTe = pool_moe.tile([E, OWN], FP32, tag="wTe")
            for t4 in range(TC4):
                wt_ps = ps_5.tile([E, P], FP32, tag="rt_ps")
                nc.tensor.transpose(wt_ps, w_tm[:, t4, :], ident)
                nc.vector.tensor_copy(wTe[:, t4 * P:(t4 + 1) * P], wt_ps)
            if debug:
                nc.sync.dma_start(dbg["d_w"], wTe)
                nc.sync.dma_start(dbg["d_logits"], logitsT)

        # ================= FFNs =================
        moe_acc = pool_moe.tile([P, KC, OWN], FP32, tag="moe_acc")

        def ffn_hidden(w1_ap, w3_ap, n_f, hid_pool, hid_tag, ps_pool, sc,
                       wscale_bc=None):
            fo = n_f // P
            hid = hid_pool.tile([P, fo, OWN], BF16, tag=hid_tag)
            for m2 in range(n_f // MCH):
                w1t = load_w(w1_ap, m2)
                w3t = load_w(w3_ap, m2)
                for ms in range(MCH // P):
                    fm = m2 * (MCH // P) + ms
                    u_ps = ps_pool.tile([P, OWN], FP32, tag="u_ps")
                    g_ps = ps_pool.tile([P, OWN], FP32, tag="g_ps")
                    for k in range(KC):
                        nc.tensor.matmul(u_ps, w1t[:, k, ms * P:(ms + 1) * P],
                                         hnT[:, k, :],
                                         start=(k == 0), stop=(k == KC - 1))
                    for k in range(KC):
                        nc.tensor.matmul(g_ps, w3t[:, k, ms * P:(ms + 1) * P],
                                         hnT[:, k, :],
                                         start=(k == 0), stop=(k == KC - 1))
                    g_sb = sc.tile([P, OWN], FP32, tag="g_sb")
                    nc.scalar.activation(g_sb, g_ps, Act.Silu)
                    if wscale_bc is None:
                        nc.vector.tensor_mul(hid[:, fm, :], u_ps, g_sb)
                    else:
                        ug = sc.tile([P, OWN], FP32, tag="ug")
                        nc.vector.tensor_mul(ug, u_ps, g_sb)
                        nc.vector.tensor_mul(hid[:, fm, :], ug, wscale_bc)
            return hid

        def ffn_out(w2_ap, hid, n_f, ps_pool, first):
            fo = n_f // P
            for m2 in range(C // MCH):
                w2ts = [load_w(w2_ap, m2, kgroup=kg)
                        for kg in range(fo // KC)]
                for ms in range(MCH // P):
                    cm = m2 * (MCH // P) + ms
                    o_ps = ps_pool.tile([P, OWN], FP32, tag="o_ps")
                    for kg, w2t in enumerate(w2ts):
                        for k in range(KC):
                            kk = kg * KC + k
                            nc.tensor.matmul(o_ps,
                                             w2t[:, k, ms * P:(ms + 1) * P],
                                             hid[:, kk, :],
                                             start=(kk == 0),
                                             stop=(kk == fo - 1))
                    if first:
                        nc.vector.tensor_copy(moe_acc[:, cm, :], o_ps)
                    else:
                        nc.vector.tensor_add(moe_acc[:, cm, :],
                                             moe_acc[:, cm, :], o_ps)
            return

        with contextlib.ExitStack() as es_p:
            ps_6 = es_p.enter_context(
                tc.tile_pool(name="ps_6", bufs=2, space="PSUM"))
            sc = es_p.enter_context(tc.tile_pool(name="sc_ffn", bufs=3))
            with contextlib.ExitStack() as es_sh:
                pool_shid = es_sh.enter_context(
                    tc.tile_pool(name="pool_shid", bufs=1))
                s_hid = ffn_hidden(s_w1, s_w3, F2, pool_shid, "s_hid", ps_6, sc)
                ffn_out(s_w2, s_hid, F2, ps_6, first=True)

            for e in range(E):
                wrow = sc.tile([1, OWN], FP32, tag="wrow")
                nc.sync.dma_start(wrow, wTe[e:e + 1, :])
                wbc = sc.tile([P, OWN], FP32, tag="wbc")
                nc.gpsimd.partition_broadcast(wbc, wrow)
                hid = ffn_hidden(e_w1[e], e_w3[e], F, hidpool, "e_hid", ps_6,
                                 sc, wscale_bc=wbc)
                ffn_out(e_w2[e], hid, F, ps_6, first=False)

            if debug:
                nc.sync.dma_start(_r(dbg["d_moe"]), moe_acc)

            # ================= final =================
            for cm in range(KC):
                o_sb = sc.tile([P, OWN], FP32, tag="o_sb")
                nc.vector.tensor_add(o_sb, hT[:, cm, :], moe_acc[:, cm, :])
                nc.sync.dma_start(_r(outT)[:, cm, :], o_sb)
        es_moe.close()


# ---------------------------------------------------------------------------
# host side
# ---------------------------------------------------------------------------
def _tile_w(w):
    """[K, M] fp32 -> [M/MCH, K/P, P, min(MCH,M)] bf16 contiguous chunks."""
    K, M = w.shape
    mch = min(MCH, M)
    mo, ko = (M + mch - 1) // mch, K // P
    t = w.reshape(ko, P, mo, mch).transpose(2, 0, 1, 3)
    return np.ascontiguousarray(t.astype(ml_dtypes.bfloat16))


def prep_in_maps(inputs):
    f32 = lambda a: np.ascontiguousarray(np.asarray(a), dtype=np.float32)
    x = f32(inputs["x"])
    ga = f32(inputs["g_attn"])[:, None]
    gm = f32(inputs["g_moe"])[:, None]
    ew1 = f32(inputs["e_w1"]) * gm[None]
    ew2 = f32(inputs["e_w2"])
    ew3 = f32(inputs["e_w3"]) * gm[None]
    shared = {
        "wq": _tile_w(f32(inputs["wq"]) * ga / np.sqrt(np.float32(HD))),
        "wkv": _tile_w(f32(inputs["wkv_down"]) * ga),
        "wk_up": _tile_w(f32(inputs["wk_up"])),
        "wv_up": _tile_w(f32(inputs["wv_up"])),
        "wo": _tile_w(f32(inputs["wo"])),
        "wr": np.ascontiguousarray(f32(inputs["wr"]) * gm
                                   / np.sqrt(np.float32(C))),
        "rb": f32(inputs["rb"]).reshape(E, 1),
        "e_w1": np.stack([_tile_w(ew1[e]) for e in range(E)]),
        "e_w2": np.stack([_tile_w(ew2[e]) for e in range(E)]),
        "e_w3": np.stack([_tile_w(ew3[e]) for e in range(E)]),
        "s_w1": _tile_w(f32(inputs["s_w1"]) * gm),
        "s_w2": _tile_w(f32(inputs["s_w2"])),
        "s_w3": _tile_w(f32(inputs["s_w3"]) * gm),
    }
    in_maps = []
    t_idx = np.arange(OWN)
    s_idx = np.arange(ROW)
    for c in range(NCORES):
        b, h = c // 2, c % 2
        m = dict(shared)
        m["x_rowT"] = np.ascontiguousarray(x[b].T)
        m["x_ownT"] = np.ascontiguousarray(x[b, h * OWN:(h + 1) * OWN].T)
        m["maskT"] = np.ascontiguousarray(
            (s_idx[:, None] <= (h * OWN + t_idx)[None, :])
            .astype(ml_dtypes.bfloat16))
        in_maps.append(m)
    return in_maps


def assemble(results):
    out = np.empty((B, T, C), np.float32)
    for c in range(NCORES):
        b, h = c // 2, c % 2
        out[b, h * OWN:(h + 1) * OWN, :] = results[c]["outT"].T
    return out


_NC_CACHE = {}


def get_nc(debug=False):
    if debug not in _NC_CACHE:
        _NC_CACHE[debug] = build_nc(debug=debug)
    return _NC_CACHE[debug]


def run(inputs, debug=False, trace=False, tmpdir=None):
    nc = get_nc(debug=debug)
    in_maps = prep_in_maps(inputs)
    res = run_bass_kernel_spmd(nc, in_maps, list(range(NCORES)),
                               trace=trace, tmpdir=tmpdir)
    return res


def kernel(**inputs):
    res = run(inputs, debug=False, trace=False)
    return assemble(res.results)

